# revision 24
# baseline (speedup 1.0000x reference)
"""BiLSTM Trainium2 Bass kernel.

Reference semantics (with the source's transpose quirk folded in):
  xtilde[j, t, d] = x[d, t, j]  (valid since B == D == 128)
  standard LSTM over xtilde for fwd and (time-reversed) bwd directions,
  gate order in weights: f, i, c, o;  z = W @ [x_t ; h], fp32.

Sharding: 8 cores = 2 direction groups x 4 batch shards (32 LSTM-batch rows
each).  The bwd direction is implemented by feeding time-reversed x and
re-reversing the output on the host, so all 8 cores run the identical SPMD
program.

The whole x shard (128 x S x 32 bf16 = 8MB = 64KB/partition) is resident in
SBUF, and the S-step recurrence is fully unrolled straight-line code (no
hardware loops, all access patterns static).

Per core, per timestep (gate column order [i, c', f, o], c' pre-scaled by 2
so sigmoid(2 z_c) = (tanh(z_c)+1)/2):
  PE : z[32,1024](psum,fp32) = x_t[128,32].T @ wx[128,1024]
                             + hT0[128,32].T @ wh0[128,1024]
                             + hT1[128,32].T @ wh1[128,1024]   (bf16 in)
       accumulated as three groups in separate psum banks
       (g0=[i,c'], f, o) so the critical consumers start early
  ACT: sig_g0, sig_f, sig_o (bf16 out) ; th = tanh(c) ; hT1 copy psum->sbuf
  DVE: m = 2*s_c - 1 ; t = s_i*m ; c = cf + t ; h_bf = s_o*th ; hT0 copy
  GPS: cf = s_f * c ; h_fp = cast(h_bf)
  PE : hT(next) = transpose(h_bf)  (2 x [32,128] -> [128,32] bf16,
       separate psum banks)
  DMA: hstage -> outseq every OUTCHUNK steps
"""

import numpy as np
import ml_dtypes

BF16 = ml_dtypes.bfloat16

B = 128
S = 1024
D = 128
H = 256
NCORES = 8
BS = 32          # LSTM-batch rows per core
OUTCHUNK = 16    # timesteps per output DMA


def build_nc(seq_len=S, bs=BS, outchunk=OUTCHUNK, with_bias=False, repeats=1):
    import contextlib
    import concourse.bass as bass
    import concourse.bacc as bacc
    import concourse.mybir as mybir
    import concourse.tile as tile

    dt = mybir.dt
    AF = mybir.ActivationFunctionType
    OP = mybir.AluOpType

    nc = bacc.Bacc("TRN2", target_bir_lowering=False, debug=False)

    G = 4 * H  # 1024 gate columns
    x_d = nc.dram_tensor("x", [B, seq_len, bs], dt.bfloat16, kind="ExternalInput")
    wx_d = nc.dram_tensor("wx", [D, G], dt.bfloat16, kind="ExternalInput")
    wh0_d = nc.dram_tensor("wh0", [128, G], dt.bfloat16, kind="ExternalInput")
    wh1_d = nc.dram_tensor("wh1", [128, G], dt.bfloat16, kind="ExternalInput")
    id_d = nc.dram_tensor("ident", [bs, bs], dt.bfloat16, kind="ExternalInput")
    if with_bias:
        bias_d = nc.dram_tensor("bias", [1, G], dt.bfloat16, kind="ExternalInput")
    outseq_d = nc.dram_tensor(
        "outseq", [bs, seq_len, H], dt.float32, kind="ExternalOutput"
    )
    hfin_d = nc.dram_tensor("hfin", [bs, H], dt.float32, kind="ExternalOutput")
    cfin_d = nc.dram_tensor("cfin", [bs, H], dt.float32, kind="ExternalOutput")

    assert seq_len % outchunk == 0 and outchunk % 2 == 0

    with tile.TileContext(nc) as tc:
        with contextlib.ExitStack() as ctx:
            const_p = ctx.enter_context(tc.tile_pool(name="const", bufs=1))
            state_p = ctx.enter_context(tc.tile_pool(name="state", bufs=1))
            s_p = ctx.enter_context(tc.tile_pool(name="s", bufs=2))
            ew_p = ctx.enter_context(tc.tile_pool(name="ew", bufs=2))
            hs_p = ctx.enter_context(tc.tile_pool(name="hs", bufs=3))
            z_p = ctx.enter_context(tc.tile_pool(name="z", bufs=2, space="PSUM"))
            tp_p = ctx.enter_context(tc.tile_pool(name="tp", bufs=2, space="PSUM"))

            # constants / weights / whole x shard, resident in SBUF
            xsb = const_p.tile([B, seq_len, bs], dt.bfloat16, tag="xsb")
            wx = const_p.tile([D, G], dt.bfloat16, tag="wx")
            wh0 = const_p.tile([128, G], dt.bfloat16, tag="wh0")
            wh1 = const_p.tile([128, G], dt.bfloat16, tag="wh1")
            ident = const_p.tile([bs, bs], dt.bfloat16, tag="ident")
            nxload = 8
            for i in range(nxload):
                sl = slice(i * seq_len // nxload, (i + 1) * seq_len // nxload)
                nc.sync.dma_start(xsb[:, sl, :], x_d[:, sl, :])
            nc.sync.dma_start(wx[:], wx_d[:])
            nc.sync.dma_start(wh0[:], wh0_d[:])
            nc.sync.dma_start(wh1[:], wh1_d[:])
            nc.sync.dma_start(ident[:], id_d[:])
            if with_bias:
                bias_sb = const_p.tile([1, G], dt.bfloat16, tag="bias")
                ones_sb = const_p.tile([1, bs], dt.bfloat16, tag="ones")
                nc.sync.dma_start(bias_sb[:], bias_d[:])
                nc.vector.memset(ones_sb[:], 1.0)
            # zero rhs for PE-warming dummy matmuls (keeps the HAM clock
            # gate at 8/8 through the per-step PE-idle stretch)
            zeros_sb = const_p.tile([bs, 512], dt.bfloat16, tag="zeros")
            nc.vector.memset(zeros_sb[:], 0.0)

            # persistent state
            c = state_p.tile([bs, H], dt.float32, tag="c")
            # hT double-buffer: even steps read slot 0 / write slot 1
            hT = [
                [state_p.tile([128, bs], dt.bfloat16, tag=f"hT{p}{h}",
                              name=f"hT{p}{h}") for h in (0, 1)]
                for p in (0, 1)
            ]
            nc.vector.memset(c[:], 0.0)
            for p in (0, 1):
                for hh in (0, 1):
                    nc.vector.memset(hT[p][hh][:], 0.0)

            h_fp_last = None
            warm_prev = [None, None]

            def step(t, hstage):
                nonlocal h_fp_last
                k = t % outchunk
                rd = t % 2          # hT slot read by this step
                wr = (t + 1) % 2    # hT slot written by this step
                xt_ap = xsb[:, t, :]

                # gate columns: [0:256]=i [256:512]=c'(x2) [512:768]=f [768:1024]=o
                # three psum groups in separate banks: g0=[i,c'] f o
                z0 = z_p.tile([bs, 512], dt.float32, tag="z0", name="z0")
                z1 = z_p.tile([bs, 256], dt.float32, tag="z1", name="z1")
                z2 = z_p.tile([bs, 256], dt.float32, tag="z2", name="z2")

                def grp(zt, lo, hi, pre):
                    """accumulate z[:, lo:hi] into zt; pre=True groups are
                    opened early (prefilled during the previous step)"""
                    st = True
                    if with_bias:
                        nc.tensor.matmul(zt, ones_sb[:], bias_sb[:, lo:hi],
                                         start=True, stop=False)
                        st = False
                    nc.tensor.matmul(zt, xt_ap, wx[:, lo:hi], start=st, stop=False)

                def rec(zt, lo, hi, stop0=False):
                    nc.tensor.matmul(zt, hT[rd][0][:], wh0[:, lo:hi],
                                     start=False, stop=False)
                    nc.tensor.matmul(zt, hT[rd][1][:], wh1[:, lo:hi],
                                     start=False, stop=True)

                grp(z0[:], 0, 512, True)
                grp(z1[:], 512, 768, True)
                # PE warmers: zero-product accumulates into the open z0
                # group, gated on the previous step's mid-chain tiles so
                # they fire inside the PE-idle stretch and keep HAM at 8/8.
                # Adding exact zeros -> numerically inert.
                if warm_prev[0] is not None:
                    m_prev, th_prev = warm_prev
                    nc.tensor.matmul(z0[:], m_prev[:, 0:bs], zeros_sb[:],
                                     start=False, stop=False)
                    nc.tensor.matmul(z0[:], th_prev[:, 0:bs], zeros_sb[:],
                                     start=False, stop=False)
                rec(z0[:], 0, 512)      # -> sig_g0 ready first
                rec(z1[:], 512, 768)    # -> sig_f
                grp(z2[:], 768, 1024, False)
                rec(z2[:], 768, 1024)   # -> sig_o (not latency critical)

                s = s_p.tile([bs, G], dt.bfloat16, tag="s", name="s")
                nc.scalar.activation(s[:, 0:512], z0[:], AF.Sigmoid)
                nc.scalar.activation(s[:, 512:768], z1[:], AF.Sigmoid)
                nc.scalar.activation(s[:, 768:1024], z2[:], AF.Sigmoid)
                si, sc, sf, so = (s[:, 0:256], s[:, 256:512], s[:, 512:768],
                                  s[:, 768:1024])

                m = ew_p.tile([bs, H], dt.bfloat16, tag="m", name="m")
                t_ = ew_p.tile([bs, H], dt.bfloat16, tag="t", name="t")
                cf = ew_p.tile([bs, H], dt.float32, tag="cf", name="cf")
                th = ew_p.tile([bs, H], dt.bfloat16, tag="th", name="th")
                h_bf = ew_p.tile([bs, H], dt.bfloat16, tag="hbf", name="hbf")
                h_fp = hstage[:, k, :]

                nc.vector.tensor_scalar(m[:], sc, 2.0, 1.0, OP.mult, OP.subtract)
                nc.gpsimd.tensor_tensor(cf[:], sf, c[:], OP.mult)
                nc.vector.tensor_tensor(t_[:], si, m[:], OP.mult)
                nc.vector.tensor_tensor(c[:], cf[:], t_[:], OP.add)
                nc.scalar.activation(th[:], c[:], AF.Tanh)
                nc.vector.tensor_tensor(h_bf[:], so, th[:], OP.mult)

                # next step's stationary operand: hT[wr] = h^T
                # separate psum banks so tp1 isn't serialized behind cp0;
                # copies split DVE/ACT to run concurrently
                tp0 = tp_p.tile([128, bs], dt.bfloat16, tag="tp0", name="tp0",
                                bufs=1)
                tp1 = tp_p.tile([128, bs], dt.bfloat16, tag="tp1", name="tp1",
                                bufs=1)
                nc.tensor.transpose(tp0[:], h_bf[:, 0:128], ident[:])
                nc.tensor.transpose(tp1[:], h_bf[:, 128:256], ident[:])
                nc.vector.tensor_copy(hT[wr][0][:], tp0[:])
                nc.scalar.copy(hT[wr][1][:], tp1[:])

                nc.gpsimd.tensor_copy(h_fp[:], h_bf[:])
                warm_prev[0], warm_prev[1] = m, th
                h_fp_last = h_fp

            def full_pass():
                for t0 in range(0, seq_len, outchunk):
                    hstage = hs_p.tile([bs, outchunk, H], dt.float32,
                                       tag="hstage", name="hstage")
                    for k in range(outchunk):
                        step(t0 + k, hstage)
                    nc.sync.dma_start(outseq_d[:, t0:t0 + outchunk, :],
                                      hstage[:])

            if repeats == 1:
                full_pass()
            else:
                # timing-only mode: run the whole pass `repeats` times so
                # device time dominates host/transfer noise
                with tc.For_i(0, repeats, 1):
                    full_pass()

            nc.sync.dma_start(hfin_d[:], h_fp_last[:])
            nc.sync.dma_start(cfin_d[:], c[:])

    nc.compile()
    return nc


def _prep_weights(W, b):
    """W: (4,H,D+H) order f,i,c,o -> wx (D,4H), wh0/wh1 (128,4H), bias (1,4H)
    in gate-column order [i, c'(x2), f, o]."""
    W = np.asarray(W, np.float32)
    b = np.asarray(b, np.float32)
    Wx = np.concatenate([W[1, :, :D], 2.0 * W[2, :, :D], W[0, :, :D], W[3, :, :D]],
                        axis=0)            # (4H, D)
    Wh = np.concatenate([W[1, :, D:], 2.0 * W[2, :, D:], W[0, :, D:], W[3, :, D:]],
                        axis=0)            # (4H, H)
    bb = np.concatenate([b[1], 2.0 * b[2], b[0], b[3]])[None, :]  # (1, 4H)
    wx = np.ascontiguousarray(Wx.T).astype(BF16)
    whT = np.ascontiguousarray(Wh.T)       # (H, 4H)
    return wx, whT[:128].astype(BF16), whT[128:].astype(BF16), bb.astype(BF16)


_NC_CACHE = {}
LAST_EXEC_NS = None
LAST_RESULTS = None


def kernel(x, W_fwd, b_fwd, W_bwd, b_bwd):
    global LAST_EXEC_NS, LAST_RESULTS
    from concourse.bass_utils import run_bass_kernel_spmd

    x = np.asarray(x, np.float32)
    with_bias = bool(np.any(np.asarray(b_fwd)) or np.any(np.asarray(b_bwd)))

    key = (S, BS, OUTCHUNK, with_bias)
    if key not in _NC_CACHE:
        _NC_CACHE[key] = build_nc(S, BS, OUTCHUNK, with_bias)
    nc = _NC_CACHE[key]

    wpack = {0: _prep_weights(W_fwd, b_fwd), 1: _prep_weights(W_bwd, b_bwd)}
    ident = np.eye(BS, dtype=BF16)
    x_bf = x.astype(BF16)                  # (B=d, S, 128=j)
    x_rev = x_bf[:, ::-1, :]

    in_maps = []
    for core in range(NCORES):
        d = core // 4
        js = slice(BS * (core % 4), BS * (core % 4 + 1))
        wx, wh0, wh1, bb = wpack[d]
        m = {
            "x": np.ascontiguousarray((x_bf if d == 0 else x_rev)[:, :, js]),
            "wx": wx, "wh0": wh0, "wh1": wh1, "ident": ident,
        }
        if with_bias:
            m["bias"] = bb
        in_maps.append(m)

    kres = run_bass_kernel_spmd(nc, in_maps, list(range(NCORES)))
    res = kres.results
    LAST_RESULTS = kres
    if kres.exec_time_ns is not None:
        LAST_EXEC_NS = kres.exec_time_ns

    out = np.empty((B, S, 2 * H), np.float32)
    h_final = np.empty((2, B, H), np.float32)
    c_final = np.empty((2, B, H), np.float32)
    for core in range(NCORES):
        d = core // 4
        js = slice(BS * (core % 4), BS * (core % 4 + 1))
        r = res[core]
        seq = np.asarray(r["outseq"], np.float32)
        if d == 0:
            out[js, :, :H] = seq
        else:
            out[js, :, H:] = seq[:, ::-1, :]
        h_final[d, js] = np.asarray(r["hfin"], np.float32)
        c_final[d, js] = np.asarray(r["cfin"], np.float32)

    return out, (h_final, c_final)
